# revision 14
# baseline (speedup 1.0000x reference)
"""GCN encoder kernel for Trainium2, SPMD across 8 NeuronCores.

Computes (reference semantics):
    x_ = P @ (x @ W1 + b1)
    h  = P @ (1.8 * l2norm_rows(x @ W2 + b2))
where P = D^-1/2 (A + I) D^-1/2 over the edge list (by destination).

Strategy (v2):
  * dinv folded into the matmul: x rows are prescaled by dinv on the host
    and the bias enters via a contract-1 matmul whose lhsT column is dinv,
    so PSUM directly holds dinv*(xW+b) for both branches.  Branch 2's
    l2-normalize is row-scale invariant, so the dinv row scale cancels
    exactly there and only the cheap phi2 = dinv/|row| rescale remains.
  * Both branches concatenated into u[N, 256] (bf16), replicated on every
    core (phase A).
  * 784 dst windows of 128 nodes are load-balance assigned: windows sorted
    by edge count, rank-group 8j spread over the 8 cores, so per-slot tile
    budgets are shared across cores (single SPMD program) with minimal
    padding.  Windows processed in pairs; one dma_gather per (pair, seg)
    halves the ~1us fixed SWDGE overhead per call.
  * Padding entries inside a call use idx 0 (dummy gather), trailing
    padding uses idx -1 which SWDGE skips entirely; the true descriptor
    count is loaded per-core at runtime into num_idxs_reg from an input
    tensor.  Slot -1 zeroes the one-hot column either way.
  * One-hot S tiles built on DVE in all-bf16 (2x/4x fast path), matmul
    segment-sum accumulated in two PSUM chains (one per window of the
    pair), final dinv[dst] scale on the Scalar engine.
"""
import sys

import numpy as np

try:
    import concourse.bass as bass  # noqa: F401
except ImportError:
    sys.path.insert(0, "/opt/trn_rl_repo")

from contextlib import ExitStack

from ml_dtypes import bfloat16

import concourse.bass as bass
import concourse.bacc as bacc
import concourse.tile as tile
from concourse import mybir
from concourse.bass_utils import run_bass_kernel_spmd

N_CORES = 8
WIN = 128
NSEG = 4
ROWG = 512   # phase-A row group (per xT load)
MSG_BUFS = 2


def _dims(n_nodes, c_in, c_out):
    nw_g = -(-n_nodes // WIN)
    nw_g = -(-nw_g // N_CORES) * N_CORES        # pad to multiple of 8 -> 784
    u_rows = nw_g * WIN
    seg = u_rows // NSEG
    assert seg % 128 == 0 and seg <= 32768
    nwin = nw_g // N_CORES
    assert nwin % 2 == 0
    return dict(
        N=n_nodes, CIN=c_in, COUT=c_out, C=2 * c_out,
        NW_G=nw_g, NWIN=nwin, NPAIR=nwin // 2,
        U_ROWS=u_rows, G=u_rows // ROWG, KCH=c_in // 128, SEG=seg,
    )


def _prep(x, edge_index, W1, b1, W2, b2):
    """Host-side sharding: degrees, window balancing, edge packing."""
    x = np.asarray(x, np.float32)
    n, c_in = x.shape
    c_out = W1.shape[1]
    d = _dims(n, c_in, c_out)
    SEG, NW_G, NPAIR = d["SEG"], d["NW_G"], d["NPAIR"]

    src = np.asarray(edge_index[0], np.int64)
    dst = np.asarray(edge_index[1], np.int64)

    deg = (np.bincount(dst, minlength=n) + 1).astype(np.float32)
    dinv = (1.0 / np.sqrt(deg)).astype(np.float32)
    dinv_pad = np.ones(d["U_ROWS"], np.float32)
    dinv_pad[:n] = dinv

    # combined edge list: real edges + self loops
    n_all = np.arange(n, dtype=np.int64)
    src_a = np.concatenate([src, n_all])
    dst_a = np.concatenate([dst, n_all])
    gw = dst_a // WIN                      # global window id
    sg = src_a // SEG                      # source segment

    # balanced window -> (core, slot) assignment
    cnt_ws = np.bincount(gw * NSEG + sg, minlength=NW_G * NSEG) \
        .reshape(NW_G, NSEG).astype(np.int64)
    order = np.argsort(-cnt_ws.sum(1), kind="stable")
    wa = order.reshape(d["NWIN"], N_CORES)         # wa[j, k] = window id
    core_of = np.empty(NW_G, np.int64)
    slot_of = np.empty(NW_G, np.int64)
    core_of[order] = np.tile(np.arange(N_CORES), d["NWIN"])
    slot_of[order] = np.repeat(np.arange(d["NWIN"]), N_CORES)

    # per (slot, seg) tile budget shared by all cores
    t_js = np.maximum(1, -(-cnt_ws[wa, :].max(axis=1) // WIN))   # [NWIN, NSEG]
    t0 = t_js[0::2]                                              # [NPAIR, NSEG]
    t1 = t_js[1::2]
    T_ps = t0 + t1
    OFF = np.zeros((NPAIR, NSEG), np.int64)
    OFF[:, 1:] = np.cumsum(T_ps, axis=1)[:, :-1]
    TTP = T_ps.sum(1)
    TTPM = int(TTP.max())
    d["TTPM"] = TTPM
    d["T_PS"] = T_ps
    d["OFF"] = OFF
    d["T0"] = t0
    d["T1"] = t1

    # order edges by (core, pair, seg, window-in-pair, src)
    k_e = core_of[gw]
    j_e = slot_of[gw]
    grp_key = (gw * NSEG + sg)           # group by (window, seg)
    order_e = np.lexsort((src_a, sg, j_e % 2, (j_e // 2), k_e))
    src_o = src_a[order_e]
    dst_o = dst_a[order_e]
    gw_o = gw[order_e]
    sg_o = sg[order_e]
    k_o = k_e[order_e]
    j_o = j_e[order_e]
    grp_o = grp_key[order_e]
    # position within each (window, seg) group (blocks are contiguous in
    # the sorted stream; pos restarts at each block boundary)
    m = grp_o.shape[0]
    is_start = np.empty(m, bool)
    is_start[0] = True
    is_start[1:] = grp_o[1:] != grp_o[:-1]
    arange_m = np.arange(m, dtype=np.int64)
    pos = arange_m - np.maximum.accumulate(np.where(is_start, arange_m, 0))

    p_o = j_o // 2
    w01 = j_o % 2
    li = w01 * t0[p_o, sg_o] * WIN + pos             # index within the call
    col = OFF[p_o, sg_o] + li // WIN                 # msg tile column

    es_buf = np.full((N_CORES, NPAIR, WIN, TTPM), -1.0, bfloat16)
    es_buf[k_o, p_o, li % WIN, col] = (dst_o % WIN).astype(bfloat16)

    idx16 = np.full((N_CORES, NPAIR, 16, TTPM * 8), -1, np.int16)
    # interior dummy padding: w0 block of each call is fully "valid" (idx 0)
    for p in range(NPAIR):
        for s in range(NSEG):
            o8 = OFF[p, s] * 8
            idx16[:, p, :, o8:o8 + t0[p, s] * 8] = 0
    idx16[k_o, p_o, li % 16, OFF[p_o, sg_o] * 8 + li // 16] = \
        (src_o - sg_o * SEG).astype(np.int16)
    idx_buf = np.tile(idx16, (1, 1, 8, 1))           # [8, NPAIR, 128, TTPM*8]

    # per-core runtime descriptor counts: w0 block full + w1 valid count
    c1 = cnt_ws[wa[1::2, :], :]                      # [NPAIR, 8, NSEG]
    cnt_reg = (t0[:, None, :] * WIN + c1).astype(np.int32)  # [NPAIR, 8, NSEG]

    # constants / features
    dr = dinv_pad.reshape(NW_G, WIN)
    dinvr = np.ascontiguousarray(dr.T)               # [128, NW_G] f32
    xs = x * dinv[:, None]
    xt = np.zeros((c_in, d["U_ROWS"]), bfloat16)
    xt[:, :n] = xs.T.astype(bfloat16)
    dvrow = np.zeros((1, d["U_ROWS"]), bfloat16)
    dvrow[0, :] = dinv_pad.astype(bfloat16)
    wc = np.concatenate([W1, W2], axis=1).astype(bfloat16)     # [CIN, C]
    brow = np.concatenate([b1, b2]).astype(bfloat16)[None, :]  # [1, C]
    iota_bc = np.tile(
        np.arange(WIN, dtype=np.float32)[None, :], (WIN, 1)).astype(bfloat16)

    in_maps = []
    for k in range(N_CORES):
        dd = np.ascontiguousarray(dr[wa[:, k], :].T)           # [128, NWIN]
        in_maps.append({
            "xt": xt,
            "dvrow": dvrow,
            "wc": wc,
            "brow": brow,
            "iota": iota_bc,
            "dinvr": dinvr,
            "dinvd": dd,
            "edi": idx_buf[k],
            "eds": es_buf[k],
            "cnt": np.ascontiguousarray(cnt_reg[:, k, :].T),   # [NSEG, NPAIR]
        })
    d["WA"] = wa
    return in_maps, d


def _build(d):
    """Emit the SPMD Bass program (identical on all cores; data differs)."""
    f32, bf16 = mybir.dt.float32, mybir.dt.bfloat16
    i16, i32 = mybir.dt.int16, mybir.dt.int32
    C, CIN, KCH = d["C"], d["CIN"], d["KCH"]
    SEG, TTPM = d["SEG"], d["TTPM"]
    T_PS, OFF, T0, T1 = d["T_PS"], d["OFF"], d["T0"], d["T1"]
    nrt = d["U_ROWS"] // 128
    co = d["COUT"]
    inv_s2 = 1.0 / (1.8 * 1.8)

    nc = bacc.Bacc("TRN2", target_bir_lowering=False, debug=False,
                   num_swdge_queues=4)
    xt_d = nc.dram_tensor("xt", [CIN, d["U_ROWS"]], bf16, kind="ExternalInput")
    dv_d = nc.dram_tensor("dvrow", [1, d["U_ROWS"]], bf16, kind="ExternalInput")
    wc_d = nc.dram_tensor("wc", [CIN, C], bf16, kind="ExternalInput")
    brow_d = nc.dram_tensor("brow", [1, C], bf16, kind="ExternalInput")
    iota_d = nc.dram_tensor("iota", [128, 128], bf16, kind="ExternalInput")
    dinvr_d = nc.dram_tensor("dinvr", [128, d["NW_G"]], f32,
                             kind="ExternalInput")
    dinvd_d = nc.dram_tensor("dinvd", [128, d["NWIN"]], f32,
                             kind="ExternalInput")
    edi_d = nc.dram_tensor("edi", [d["NPAIR"], 128, TTPM * 8], i16,
                           kind="ExternalInput")
    eds_d = nc.dram_tensor("eds", [d["NPAIR"], 128, TTPM], bf16,
                           kind="ExternalInput")
    cnt_d = nc.dram_tensor("cnt", [NSEG, d["NPAIR"]], i32,
                           kind="ExternalInput")
    out_d = nc.dram_tensor("out", [d["NWIN"] * 128, C], f32,
                           kind="ExternalOutput")
    u_d = nc.dram_tensor("u", [d["U_ROWS"], C], bf16)  # internal scratch

    with ExitStack() as ctx:
        tc = ctx.enter_context(tile.TileContext(nc))
        const_p = ctx.enter_context(tc.tile_pool(name="const", bufs=1))
        xa_p = ctx.enter_context(tc.tile_pool(name="xa", bufs=4))
        sq_p = ctx.enter_context(tc.tile_pool(name="sq", bufs=4))
        col_p = ctx.enter_context(tc.tile_pool(name="col", bufs=16))
        ua_p = ctx.enter_context(tc.tile_pool(name="ua", bufs=6))
        ed_p = ctx.enter_context(tc.tile_pool(name="ed", bufs=3))
        msg_p = ctx.enter_context(tc.tile_pool(name="msg", bufs=MSG_BUFS))
        s_p = ctx.enter_context(tc.tile_pool(name="s", bufs=56))
        out_p = ctx.enter_context(tc.tile_pool(name="o", bufs=4))
        psa_p = ctx.enter_context(tc.tile_pool(name="psa", bufs=4, space="PSUM"))
        psb_p = ctx.enter_context(tc.tile_pool(name="psb", bufs=2, space="PSUM"))

        # constants
        wc_t = [const_p.tile([128, C], bf16, name=f"wct{kc}", tag=f"wc{kc}")
                for kc in range(KCH)]
        for kc in range(KCH):
            nc.sync.dma_start(out=wc_t[kc][:], in_=wc_d[kc * 128:(kc + 1) * 128, :])
        brow_t = const_p.tile([1, C], bf16)
        nc.sync.dma_start(out=brow_t[:], in_=brow_d[:, :])
        iota_t = const_p.tile([128, 128], bf16)
        nc.sync.dma_start(out=iota_t[:], in_=iota_d[:, :])
        dinvr_t = const_p.tile([128, nrt], f32)
        nc.sync.dma_start(out=dinvr_t[:], in_=dinvr_d[:, :])
        dinvd_t = const_p.tile([128, d["NWIN"]], f32)
        nc.sync.dma_start(out=dinvd_t[:], in_=dinvd_d[:, :])
        cnt_t = const_p.tile([NSEG, d["NPAIR"]], i32)
        nc.sync.dma_start(out=cnt_t[:], in_=cnt_d[:, :])
        eps_t = const_p.tile([128, 1], f32)
        nc.vector.memset(eps_t[:], 1e-24)
        zeros_t = const_p.tile([128, 128], bf16)
        nc.vector.memset(zeros_t[:], 0.0)
        zeros32_t = const_p.tile([128, 128], f32)
        nc.vector.memset(zeros32_t[:], 0.0)

        creg = nc.gpsimd.alloc_register("gcnt")

        # ---- phase A: u[r] = [dinv*(x@W1+b1) | dinv*1.8*l2n(x@W2+b2)] ----
        for g in range(d["G"]):
            xg = [xa_p.tile([128, ROWG], bf16, name=f"xg{kc}", tag=f"xg{kc}")
                  for kc in range(KCH)]
            for kc in range(KCH):
                nc.sync.dma_start(
                    out=xg[kc][:],
                    in_=xt_d[kc * 128:(kc + 1) * 128, g * ROWG:(g + 1) * ROWG])
            dvg = xa_p.tile([1, ROWG], bf16, tag="dvg")
            nc.sync.dma_start(
                out=dvg[:], in_=dv_d[0:1, g * ROWG:(g + 1) * ROWG])
            for jj in range(ROWG // 128):
                rt = g * (ROWG // 128) + jj
                ps = psa_p.tile([128, C], f32)
                nc.tensor.matmul(
                    ps[:], lhsT=dvg[:, jj * 128:(jj + 1) * 128],
                    rhs=brow_t[:], start=True, stop=False)
                for kc in range(KCH):
                    nc.tensor.matmul(
                        ps[:], lhsT=xg[kc][:, jj * 128:(jj + 1) * 128],
                        rhs=wc_t[kc][:], start=False, stop=(kc == KCH - 1))
                sq_t = sq_p.tile([128, co], f32)
                s_col = col_p.tile([128, 1], f32, tag="scol")
                nc.scalar.activation(
                    out=sq_t[:], in_=ps[:, co:],
                    func=mybir.ActivationFunctionType.Square,
                    accum_out=s_col[:])
                nrm = col_p.tile([128, 1], f32, tag="nrm")
                nc.scalar.activation(
                    out=nrm[:], in_=s_col[:],
                    func=mybir.ActivationFunctionType.Sqrt,
                    bias=eps_t[:], scale=inv_s2)
                rn = col_p.tile([128, 1], f32, tag="rn")
                nc.vector.reciprocal(out=rn[:], in_=nrm[:])
                phi2 = col_p.tile([128, 1], f32, tag="phi2")
                nc.vector.tensor_tensor(
                    out=phi2[:], in0=rn[:], in1=dinvr_t[:, rt:rt + 1],
                    op=mybir.AluOpType.mult)
                u_t = ua_p.tile([128, C], bf16)
                nc.vector.tensor_tensor(
                    out=u_t[:, :co], in0=ps[:, :co], in1=zeros32_t[:],
                    op=mybir.AluOpType.add)
                nc.scalar.activation(
                    out=u_t[:, co:], in_=ps[:, co:],
                    func=mybir.ActivationFunctionType.Copy,
                    bias=0.0, scale=phi2[:])
                nc.sync.dma_start(
                    out=u_d[rt * 128:(rt + 1) * 128, :], in_=u_t[:])

        # ---- phase B: per window pair, gather + one-hot matmul ----
        for p in range(d["NPAIR"]):
            ttp = int(T_PS[p].sum())
            ei_t = ed_p.tile([128, TTPM * 8], i16, tag="ei")
            nc.sync.dma_start(out=ei_t[:, :ttp * 8], in_=edi_d[p, :, :ttp * 8])
            es_t = ed_p.tile([128, TTPM], bf16, tag="es")
            nc.sync.dma_start(out=es_t[:, :ttp], in_=eds_d[p, :, :ttp])
            msg_t = msg_p.tile([128, TTPM, C], bf16, tag="msg")
            if p < MSG_BUFS:
                # cold-start wipe: rows skipped by the gather read stale
                # SBUF, which must be finite for the zero one-hot column
                nc.gpsimd.memset(msg_t[:], 0.0)
            for s in range(NSEG):
                o0 = int(OFF[p, s])
                tps = int(T_PS[p, s])
                nc.gpsimd.reg_load(creg, cnt_t[s:s + 1, p:p + 1])
                nc.gpsimd.dma_gather(
                    out_ap=msg_t[:, o0:o0 + tps, :],
                    in_ap=u_d[s * SEG:(s + 1) * SEG, :],
                    idxs_ap=ei_t[:, o0 * 8:(o0 + tps) * 8],
                    num_idxs=tps * 128,
                    num_idxs_reg=creg,
                    elem_size=C,
                    single_packet=False,
                    queue_num=s)
            ps0 = psb_p.tile([128, C], f32, name="ps0", tag="ps0")
            ps1 = psb_p.tile([128, C], f32, name="ps1", tag="ps1")
            tiles0 = [int(OFF[p, s]) + i
                      for s in range(NSEG) for i in range(int(T0[p, s]))]
            tiles1 = [int(OFF[p, s]) + int(T0[p, s]) + i
                      for s in range(NSEG) for i in range(int(T1[p, s]))]
            # interleave the two windows' chains for PE/PSUM ILP
            seqs = []
            i0 = i1 = 0
            while i0 < len(tiles0) or i1 < len(tiles1):
                if i0 < len(tiles0):
                    seqs.append((tiles0[i0], 0)); i0 += 1
                if i1 < len(tiles1):
                    seqs.append((tiles1[i1], 1)); i1 += 1
            n0, n1 = len(tiles0), len(tiles1)
            c0 = c1 = 0
            for t, chain in seqs:
                s_t = s_p.tile([128, 128], bf16)
                nc.vector.scalar_tensor_tensor(
                    out=s_t[:], in0=iota_t[:], scalar=es_t[:, t:t + 1],
                    in1=zeros_t[:], op0=mybir.AluOpType.subtract,
                    op1=mybir.AluOpType.is_equal)
                if chain == 0:
                    nc.tensor.matmul(
                        ps0[:], lhsT=s_t[:], rhs=msg_t[:, t, :],
                        start=(c0 == 0), stop=(c0 == n0 - 1))
                    c0 += 1
                else:
                    nc.tensor.matmul(
                        ps1[:], lhsT=s_t[:], rhs=msg_t[:, t, :],
                        start=(c1 == 0), stop=(c1 == n1 - 1))
                    c1 += 1
            for w, psw in ((2 * p, ps0), (2 * p + 1, ps1)):
                o_t = out_p.tile([128, C], f32)
                nc.scalar.activation(
                    out=o_t[:], in_=psw[:],
                    func=mybir.ActivationFunctionType.Copy,
                    bias=0.0, scale=dinvd_t[:, w:w + 1])
                nc.sync.dma_start(
                    out=out_d[w * 128:(w + 1) * 128, :], in_=o_t[:])

    nc.compile()
    return nc


def _run(in_maps, d, trace=False):
    nc = _build(d)
    res = run_bass_kernel_spmd(
        nc, in_maps, core_ids=list(range(N_CORES)), trace=trace)
    co = d["COUT"]
    wa = d["WA"]
    full = np.empty((d["U_ROWS"], d["C"]), np.float32)
    rows = (wa.T[:, :, None] * 128 +
            np.arange(128, dtype=np.int64)[None, None, :])  # [8, NWIN, 128]
    for k in range(N_CORES):
        full[rows[k].reshape(-1)] = res.results[k]["out"]
    x_ = np.ascontiguousarray(full[:d["N"], :co])
    h = np.ascontiguousarray(full[:d["N"], co:])
    return (h, x_), res


def kernel(x, edge_index, W1, b1, W2, b2):
    in_maps, d = _prep(x, edge_index, W1, b1, W2, b2)
    (h, x_), _ = _run(in_maps, d, trace=False)
    return (h, x_)
